# revision 11
# baseline (speedup 1.0000x reference)
"""Trainium2 Bass kernel for nn_AUCDomainAdapation (AUC domain-adaptation loss).

Contract: kernel(**inputs) takes the FULL unsharded inputs of reference.setup_inputs()
and returns the same structure as reference.reference(**inputs): a tuple
(0.25*empirical, transfer) of float32 scalars.

Math (validated vs reference to ~1e-7):
  For sample i with label c_i, the masked pairwise sum per term reduces to
    sum_j M_ij * L(Q_ij),  Q_ij = q0_i + (lhs-scale) * B'_ij
  where B'_ij = (Y_shard @ pTm)_ij and pTm[c, j] = p[j, c] * [label_j != c]
  (class-major softmax probs with same-label columns zeroed).  Masked entries
  hit B' = 0 exactly, so the unmasked row-sum minus n_{c_i} * L(q0_i) is the
  masked sum.  L(Q) = softplus(-(Q)) + softplus(Q + 2*eps)
              = ln(c0 + e^eps * (w + 1/w)),  w = e^{Q+eps}, c0 = 1 + e^{2 eps}.

Sharding: row-shard the 1024 samples over 8 cores (128 rows each).  Inputs are
rotated per-core along the sample axis so a single SPMD program (slice [0:128])
serves all cores; per-core scalar partials are summed on the host.
"""

import numpy as np
from contextlib import ExitStack

import concourse.bass as bass
import concourse.bacc as bacc
import concourse.tile as tile
from concourse import mybir
from concourse.bass_utils import run_bass_kernel_spmd
from concourse._compat import with_exitstack

F32 = mybir.dt.float32
AF = mybir.ActivationFunctionType
ALU = mybir.AluOpType

N = 1024          # samples
C = 64            # classes
SH = 128          # rows per core
NCORES = 8
EPS = 0.05
PHASE = 99  # debug: stop kernel after this phase
C0 = float(1.0 + np.exp(2 * EPS))   # 1 + e^{2eps}
SE = float(np.exp(EPS))             # e^{eps}
CHUNK = 512                         # matmul moving free-dim max

INPUT_SPECS = {
    "yT_sx":   [128, N],   # [y_s^T ; y_s_adv^T] class-major stack
    "yT_tx":   [128, N],   # [y_t^T ; y_t_adv^T]
    "yta_nat": [128, 512], # y_t_adv natural, j=(a,p): [p, a*64+c]
    "y4_nat":  [128, 256], # shard rows: [y_s | y_sa | y_t | y_ta]
    "lab_row": [1, N],     # labels_s as f32 row
    "lab_sh":  [128, 1],   # shard labels f32
    "iota_mod": [128, 1],  # p mod 64
    "ident":   [128, 128], # identity matrix
    "lhA":     [128, 4],   # colsum lhsT: source stack -> rows 0/1
    "lhB":     [128, 4],   # colsum lhsT: target stack -> rows 2/3
    "lbS":     [4, 128],   # rb broadcast lhsT: rows 0/1 -> partitions lo/hi
    "lbT":     [4, 128],   # rb broadcast lhsT: rows 2/3 -> partitions lo/hi
}


@with_exitstack
def _body(ctx: ExitStack, tc: "tile.TileContext", ins: dict, out2: "bass.AP", beta_t: float):
    nc = tc.nc
    sb = ctx.enter_context(tc.tile_pool(name="sb", bufs=1))
    sw = ctx.enter_context(tc.tile_pool(name="sw", bufs=2))   # loss scratch
    psT = ctx.enter_context(tc.tile_pool(name="psT", bufs=2, space="PSUM"))
    psP = ctx.enter_context(tc.tile_pool(name="psP", bufs=2, space="PSUM"))

    # ---- input loads ------------------------------------------------------
    t_ysx = sb.tile([128, N], F32)
    t_ytx = sb.tile([128, N], F32)
    t_ytan = sb.tile([128, 512], F32)
    t_y4 = sb.tile([128, 256], F32)
    t_lab1 = sb.tile([1, N], F32)
    t_labsh = sb.tile([128, 1], F32)
    t_iotam = sb.tile([128, 1], F32)
    t_ident = sb.tile([128, 128], F32)
    for t, name in ((t_ysx, "yT_sx"), (t_ytx, "yT_tx"), (t_ytan, "yta_nat"),
                    (t_y4, "y4_nat"), (t_lab1, "lab_row"), (t_labsh, "lab_sh"),
                    (t_iotam, "iota_mod"), (t_ident, "ident")):
        nc.sync.dma_start(t[:], ins[name][:])

    # ---- small constants --------------------------------------------------
    t_epsb = sb.tile([128, 1], F32)
    nc.gpsimd.memset(t_epsb[:], EPS)
    t_c0b = sb.tile([128, 1], F32)
    nc.gpsimd.memset(t_c0b[:], C0)
    t_ones = sb.tile([128, 1], F32)
    nc.gpsimd.memset(t_ones[:], 1.0)
    # constant matmul patterns from host
    t_lhA = sb.tile([128, 4], F32)
    t_lhB = sb.tile([128, 4], F32)
    t_lbS = sb.tile([4, 128], F32)
    t_lbT = sb.tile([4, 128], F32)
    for t, name in ((t_lhA, "lhA"), (t_lhB, "lhB"), (t_lbS, "lbS"), (t_lbT, "lbT")):
        nc.sync.dma_start(t[:], ins[name][:])

    # ---- source one-hot stack + counts ------------------------------------
    t_lab128 = sb.tile([128, N], F32)
    nc.gpsimd.partition_broadcast(t_lab128[:], t_lab1[0:1, :])
    t_YTsx = sb.tile([128, N], F32)
    t_ns = sb.tile([128, 1], F32)
    nc.vector.tensor_scalar(t_YTsx[:], t_lab128[:], t_iotam[:], None,
                            op0=ALU.is_equal, op1=ALU.add, accum_out=t_ns[:])

    if PHASE < 2:
        t_o2dbg = sb.tile([1, 2], F32, tag="dbg")
        nc.vector.tensor_copy(t_o2dbg[:], t_YTsx[0:1, 0:2])
        nc.sync.dma_start(out2[:], t_o2dbg[:])
        return
    # ---- exps of the class-major stacks ------------------------------------
    t_Exs = sb.tile([128, N], F32)
    nc.scalar.activation(t_Exs[:], t_ysx[:], AF.Exp)
    t_Ext = sb.tile([128, N], F32)
    nc.scalar.activation(t_Ext[:], t_ytx[:], AF.Exp)

    # ---- softmax denominators (column sums) & reciprocals ------------------
    p_cs = psP.tile([4, N], F32, tag="prep")
    for ch in range(2):
        s = slice(ch * CHUNK, (ch + 1) * CHUNK)
        nc.tensor.matmul(p_cs[:, s], t_lhA[:], t_Exs[:, s], start=True, stop=False)
        nc.tensor.matmul(p_cs[:, s], t_lhB[:], t_Ext[:, s], start=False, stop=True)
    t_R4 = sb.tile([4, N], F32)
    nc.vector.reciprocal(t_R4[:], p_cs[:])

    if PHASE < 3:
        t_o2dbg = sb.tile([1, 2], F32, tag="dbg")
        nc.vector.tensor_copy(t_o2dbg[:], t_R4[0:1, 0:2])
        nc.sync.dma_start(out2[:], t_o2dbg[:])
        return
    # ---- masked-normalizer RM and masked probs psTm: source ----------------
    p_rbs = psP.tile([128, N], F32, tag="prep")
    for ch in range(2):
        s = slice(ch * CHUNK, (ch + 1) * CHUNK)
        nc.tensor.matmul(p_rbs[:, s], t_lbS[:], t_R4[:, s], start=True, stop=True)
    t_RMs = sb.tile([128, N], F32)
    nc.vector.scalar_tensor_tensor(t_RMs[:], t_YTsx[:], 0.0, p_rbs[:],
                                   op0=ALU.is_equal, op1=ALU.mult)
    t_PTs_lo = sb.tile([64, N], F32)
    nc.vector.tensor_tensor(t_PTs_lo[:], t_Exs[0:64, :], t_RMs[0:64, :], op=ALU.mult)
    t_PTs_hi = sb.tile([64, N], F32)
    nc.vector.tensor_tensor(t_PTs_hi[:], t_Exs[64:128, :], t_RMs[64:128, :], op=ALU.mult)

    if PHASE < 4:
        t_o2dbg = sb.tile([1, 2], F32, tag="dbg")
        nc.vector.tensor_copy(t_o2dbg[:], t_PTs_lo[0:1, 0:2])
        nc.sync.dma_start(out2[:], t_o2dbg[:])
        return
    # ---- target pseudo-labels (one-hot argmax of y_t_adv), class-major -----
    t_m8 = sb.tile([128, 8], F32)
    ytan_v = t_ytan[:].rearrange("p (a c) -> p a c", c=64)
    nc.vector.tensor_reduce(t_m8[:], ytan_v, axis=mybir.AxisListType.X, op=ALU.max)
    t_oh = sb.tile([128, 512], F32)
    nc.vector.tensor_tensor(t_oh[:].rearrange("p (a c) -> p a c", c=64), ytan_v,
                            t_m8[:, :, None].broadcast_to((128, 8, 64)),
                            op=ALU.is_equal)
    p_yt = psP.tile([64, N], F32, tag="prep")
    for a in range(8):
        nc.tensor.transpose(p_yt[:, a * 128:(a + 1) * 128],
                            t_oh[:, a * 64:(a + 1) * 64], t_ident[:])
    t_YtT = sb.tile([64, N], F32)
    t_nt = sb.tile([64, 1], F32)
    nc.vector.tensor_scalar(t_YtT[:], p_yt[:], 0.0, None,
                            op0=ALU.add, op1=ALU.add, accum_out=t_nt[:])

    if PHASE < 5:
        t_o2dbg = sb.tile([1, 2], F32, tag="dbg")
        nc.vector.tensor_copy(t_o2dbg[:], t_YtT[0:1, 0:2])
        nc.sync.dma_start(out2[:], t_o2dbg[:])
        return
    # ---- masked probs psTm: target -----------------------------------------
    p_rbt = psP.tile([128, N], F32, tag="prep")
    for ch in range(2):
        s = slice(ch * CHUNK, (ch + 1) * CHUNK)
        nc.tensor.matmul(p_rbt[:, s], t_lbT[:], t_R4[:, s], start=True, stop=True)
    t_RMt = sb.tile([128, N], F32)
    nc.vector.scalar_tensor_tensor(t_RMt[0:64, :], t_YtT[:], 0.0, p_rbt[0:64, :],
                                   op0=ALU.is_equal, op1=ALU.mult)
    nc.vector.scalar_tensor_tensor(t_RMt[64:128, :], t_YtT[:], 0.0, p_rbt[64:128, :],
                                   op0=ALU.is_equal, op1=ALU.mult)
    t_PTt_lo = sb.tile([64, N], F32)
    nc.vector.tensor_tensor(t_PTt_lo[:], t_Ext[0:64, :], t_RMt[0:64, :], op=ALU.mult)
    t_PTt_hi = sb.tile([64, N], F32)
    nc.vector.tensor_tensor(t_PTt_hi[:], t_Ext[64:128, :], t_RMt[64:128, :], op=ALU.mult)

    if PHASE < 6:
        t_o2dbg = sb.tile([1, 2], F32, tag="dbg")
        nc.vector.tensor_copy(t_o2dbg[:], t_PTt_lo[0:1, 0:2])
        nc.sync.dma_start(out2[:], t_o2dbg[:])
        return
    # ---- per-class fac weights, gathered to shard rows ----------------------
    def fac_of(n_ap):
        t1 = sb.tile([64, 1], F32, tag="fac_t1")
        nc.vector.tensor_scalar(t1[:], n_ap, -1.0, float(N), op0=ALU.mult, op1=ALU.add)
        t2 = sb.tile([64, 1], F32, tag="fac_t2")
        nc.vector.tensor_tensor(t2[:], t1[:], n_ap, op=ALU.mult)   # n*(N-n)
        t3 = sb.tile([64, 1], F32, tag="fac_t3")
        nc.vector.tensor_scalar(t3[:], t2[:], 1.0, None, op0=ALU.max)
        rec = sb.tile([64, 1], F32, tag="fac_rec")
        nc.vector.reciprocal(rec[:], t3[:])
        g = sb.tile([64, 1], F32, tag="fac_g")
        nc.vector.tensor_scalar(g[:], t2[:], 0.5, None, op0=ALU.is_gt)
        fac = sb.tile([64, 1], F32, tag="fac_out")
        nc.vector.tensor_tensor(fac[:], rec[:], g[:], op=ALU.mult)
        return fac

    fac_s = fac_of(t_ns[0:64, :])
    fac_t = fac_of(t_nt[:])

    t_rhs_s = sb.tile([64, 4], F32)
    nc.vector.tensor_scalar(t_rhs_s[:, 0:1], fac_s[:], 0.25, None, op0=ALU.mult)
    nc.vector.tensor_scalar(t_rhs_s[:, 1:2], fac_s[:], -0.5, None, op0=ALU.mult)
    nc.vector.tensor_copy(t_rhs_s[:, 2:3], t_ns[0:64, :])
    nc.vector.tensor_copy(t_rhs_s[:, 3:4], t_ns[0:64, :])
    t_rhs_t = sb.tile([64, 2], F32)
    nc.vector.tensor_scalar(t_rhs_t[:, 0:1], fac_t[:], 0.25 * beta_t, None, op0=ALU.mult)
    nc.vector.tensor_copy(t_rhs_t[:, 1:2], t_nt[:])

    p_gs = psP.tile([128, 4], F32, tag="prep")
    nc.tensor.matmul(p_gs[:], t_YTsx[0:64, 0:SH], t_rhs_s[:], start=True, stop=True)
    t_W3 = sb.tile([128, 3], F32)
    t_N3 = sb.tile([128, 3], F32)
    nc.vector.tensor_copy(t_W3[:, 0:2], p_gs[:, 0:2])
    nc.vector.tensor_copy(t_N3[:, 0:2], p_gs[:, 2:4])
    p_gt = psP.tile([128, 2], F32, tag="prep")
    nc.tensor.matmul(p_gt[:], t_YtT[:, 0:SH], t_rhs_t[:], start=True, stop=True)
    nc.vector.tensor_copy(t_W3[:, 2:3], p_gt[:, 0:1])
    nc.vector.tensor_copy(t_N3[:, 2:3], p_gt[:, 1:2])

    if PHASE < 7:
        t_o2dbg = sb.tile([1, 2], F32, tag="dbg")
        nc.vector.tensor_copy(t_o2dbg[:], t_W3[0:1, 0:2])
        nc.sync.dma_start(out2[:], t_o2dbg[:])
        return
    # ---- per-row selected logits / softmax denoms (natural layout) ---------
    t_iotaf = sb.tile([128, 64], F32)
    nc.gpsimd.iota(t_iotaf[:], pattern=[[1, 64]], base=0, channel_multiplier=0,
                   allow_small_or_imprecise_dtypes=True)
    t_Ysn = sb.tile([128, 64], F32)
    nc.vector.tensor_scalar(t_Ysn[:], t_iotaf[:], t_labsh[:], None, op0=ALU.is_equal)
    t_Y4n = sb.tile([128, 256], F32)
    nc.vector.tensor_copy(t_Y4n[:, 0:64], t_Ysn[:])
    nc.vector.tensor_copy(t_Y4n[:, 64:128], t_Ysn[:])
    nc.vector.tensor_copy(t_Y4n[:, 128:192], t_oh[:, 0:64])
    nc.vector.tensor_copy(t_Y4n[:, 192:256], t_oh[:, 0:64])
    t_scr = sb.tile([128, 256], F32)
    nc.vector.tensor_tensor(t_scr[:], t_Y4n[:], t_y4[:], op=ALU.mult)
    t_sel4 = sb.tile([128, 4], F32)
    nc.vector.tensor_reduce(t_sel4[:], t_scr[:].rearrange("p (a c) -> p a c", c=64),
                            axis=mybir.AxisListType.X, op=ALU.add)
    # shard softmax reciprocals: transpose R4[:, 0:128] -> [128, 4]
    p_r4t = psP.tile([128, 4], F32, tag="prep")
    nc.tensor.transpose(p_r4t[:], t_R4[:, 0:SH], t_ident[0:4, 0:4])
    t_es4 = sb.tile([128, 4], F32)
    nc.scalar.activation(t_es4[:], t_sel4[:], AF.Exp)
    t_a4 = sb.tile([128, 4], F32)
    nc.vector.tensor_tensor(t_a4[:], t_es4[:], p_r4t[:], op=ALU.mult)

    if PHASE < 8:
        t_o2dbg = sb.tile([1, 2], F32, tag="dbg")
        nc.vector.tensor_copy(t_o2dbg[:], t_a4[0:1, 0:2])
        nc.sync.dma_start(out2[:], t_o2dbg[:])
        return
    # ---- per-term q0 row constants and Exp biases ---------------------------
    t_q0 = sb.tile([128, 3], F32)
    # emp: q0 = 4 - eps - 4 a
    nc.vector.tensor_scalar(t_q0[:, 0:1], t_a4[:, 0:1], -4.0, 4.0 - EPS,
                            op0=ALU.mult, op1=ALU.add)
    # src: q0 = 2 (aa - a) - eps
    t_u = sb.tile([128, 1], F32, tag="u1")
    nc.vector.tensor_tensor(t_u[:], t_a4[:, 1:2], t_a4[:, 0:1], op=ALU.subtract)
    nc.vector.tensor_scalar(t_q0[:, 1:2], t_u[:], 2.0, -EPS, op0=ALU.mult, op1=ALU.add)
    # tgt: q0 = 2 (ata - at) - eps
    t_u2 = sb.tile([128, 1], F32, tag="u2")
    nc.vector.tensor_tensor(t_u2[:], t_a4[:, 3:4], t_a4[:, 2:3], op=ALU.subtract)
    nc.vector.tensor_scalar(t_q0[:, 2:3], t_u2[:], 2.0, -EPS, op0=ALU.mult, op1=ALU.add)
    t_bexp = sb.tile([128, 3], F32)
    nc.vector.tensor_scalar(t_bexp[:], t_q0[:], EPS, None, op0=ALU.add)

    # ---- L0 corrections (loss at B'=0) --------------------------------------
    t_w0 = sb.tile([128, 3], F32)
    nc.scalar.activation(t_w0[:], t_q0[:], AF.Exp, bias=t_epsb[:], scale=1.0)
    t_wi0 = sb.tile([128, 3], F32)
    nc.vector.reciprocal(t_wi0[:], t_w0[:])
    t_z0 = sb.tile([128, 3], F32)
    nc.vector.tensor_tensor(t_z0[:], t_w0[:], t_wi0[:], op=ALU.add)
    t_L0 = sb.tile([128, 3], F32)
    nc.scalar.activation(t_L0[:], t_z0[:], AF.Ln, bias=t_c0b[:], scale=SE)
    t_corr = sb.tile([128, 3], F32)
    nc.vector.tensor_tensor(t_corr[:], t_L0[:], t_N3[:], op=ALU.mult)

    if PHASE < 9:
        t_o2dbg = sb.tile([1, 2], F32, tag="dbg")
        nc.vector.tensor_copy(t_o2dbg[:], t_corr[0:1, 0:2])
        nc.sync.dma_start(out2[:], t_o2dbg[:])
        return
    # ---- big per-term matmuls + loss ----------------------------------------
    t_lhemp = sb.tile([64, 128], F32)
    nc.vector.tensor_scalar(t_lhemp[:], t_YTsx[0:64, 0:SH], 4.0, None, op0=ALU.mult)
    t_lh2s = sb.tile([64, 128], F32)
    nc.vector.tensor_scalar(t_lh2s[:], t_YTsx[0:64, 0:SH], 2.0, None, op0=ALU.mult)
    t_lhm2s = sb.tile([64, 128], F32)
    nc.vector.tensor_scalar(t_lhm2s[:], t_YTsx[0:64, 0:SH], -2.0, None, op0=ALU.mult)
    t_lh2t = sb.tile([64, 128], F32)
    nc.vector.tensor_scalar(t_lh2t[:], t_YtT[:, 0:SH], 2.0, None, op0=ALU.mult)
    t_lhm2t = sb.tile([64, 128], F32)
    nc.vector.tensor_scalar(t_lhm2t[:], t_YtT[:, 0:SH], -2.0, None, op0=ALU.mult)

    t_as = sb.tile([128, 3], F32)
    terms = [
        ("emp", t_lhemp, None, t_PTs_lo, None, 0),
        ("src", t_lh2s, t_lhm2s, t_PTs_lo, t_PTs_hi, 1),
        ("tgt", t_lh2t, t_lhm2t, t_PTt_lo, t_PTt_hi, 2),
    ]
    if PHASE == 91:
        terms = terms[:1]
    elif PHASE == 92:
        terms = terms[:2]
    for name, lh_a, lh_b, pt_a, pt_b, col in terms:
        p_T = psT.tile([128, N], F32, tag="T")
        for ch in range(2):
            s = slice(ch * CHUNK, (ch + 1) * CHUNK)
            if lh_b is not None:
                nc.tensor.matmul(p_T[:, s], lh_a[:], pt_a[:, s], start=True, stop=False)
                nc.tensor.matmul(p_T[:, s], lh_b[:], pt_b[:, s], start=False, stop=True)
            else:
                nc.tensor.matmul(p_T[:, s], lh_a[:], pt_a[:, s], start=True, stop=True)
        t_w = sw.tile([128, N], F32, tag="w")
        nc.scalar.activation(t_w[:], p_T[:], AF.Exp, bias=t_bexp[:, col:col + 1], scale=1.0)
        t_wi = sw.tile([128, N], F32, tag="wi")
        nc.vector.reciprocal(t_wi[:], t_w[:])
        t_z = sw.tile([128, N], F32, tag="z")
        nc.vector.tensor_tensor(t_z[:], t_w[:], t_wi[:], op=ALU.add)
        t_l = sw.tile([128, N], F32, tag="l")
        nc.scalar.activation(t_l[:], t_z[:], AF.Ln, bias=t_c0b[:], scale=SE,
                             accum_out=t_as[:, col:col + 1])

    if PHASE < 10:
        t_o2dbg = sb.tile([1, 2], F32, tag="dbg")
        nc.vector.tensor_copy(t_o2dbg[:], t_as[0:1, 0:2])
        nc.sync.dma_start(out2[:], t_o2dbg[:])
        return
    # ---- final reduction -----------------------------------------------------
    t_r3 = sb.tile([128, 3], F32)
    nc.vector.tensor_tensor(t_r3[:], t_as[:], t_corr[:], op=ALU.subtract)
    t_wr3 = sb.tile([128, 3], F32)
    nc.vector.tensor_tensor(t_wr3[:], t_W3[:], t_r3[:], op=ALU.mult)
    p_fin = psP.tile([1, 3], F32, tag="prep")
    nc.tensor.matmul(p_fin[:], t_ones[:], t_wr3[:], start=True, stop=True)
    t_fin = sb.tile([1, 3], F32)
    nc.vector.tensor_copy(t_fin[:], p_fin[:])
    t_o = sb.tile([1, 2], F32)
    nc.vector.tensor_copy(t_o[:, 0:1], t_fin[:, 0:1])
    nc.vector.tensor_tensor(t_o[:, 1:2], t_fin[:, 1:2], t_fin[:, 2:3], op=ALU.add)
    nc.sync.dma_start(out2[:], t_o[:])


_NC_CACHE = {}


def _get_nc(beta_t: float):
    key = beta_t
    if key in _NC_CACHE:
        return _NC_CACHE[key]
    nc = bacc.Bacc("TRN2", target_bir_lowering=False, debug=False, num_devices=NCORES)
    ins = {name: nc.dram_tensor(name, shape, F32, kind="ExternalInput").ap()
           for name, shape in INPUT_SPECS.items()}
    out2 = nc.dram_tensor("out2", [1, 2], F32, kind="ExternalOutput").ap()
    with tile.TileContext(nc) as tc:
        _body(tc, ins, out2, beta_t)
    nc.compile()
    _NC_CACHE[key] = nc
    return nc


def make_in_maps(y_s, y_s_adv, labels_s, y_t, y_t_adv):
    lab = np.asarray(labels_s).astype(np.float32)
    iota_mod = (np.arange(128) % 64).astype(np.float32).reshape(128, 1)
    ident = np.eye(128, dtype=np.float32)
    lhA = np.zeros((128, 4), np.float32); lhA[0:64, 0] = 1.0; lhA[64:128, 1] = 1.0
    lhB = np.zeros((128, 4), np.float32); lhB[0:64, 2] = 1.0; lhB[64:128, 3] = 1.0
    lbS = np.zeros((4, 128), np.float32); lbS[0, 0:64] = 1.0; lbS[1, 64:128] = 1.0
    lbT = np.zeros((4, 128), np.float32); lbT[2, 0:64] = 1.0; lbT[3, 64:128] = 1.0
    arrs = [np.ascontiguousarray(np.asarray(a, dtype=np.float32))
            for a in (y_s, y_s_adv, y_t, y_t_adv)]
    in_maps = []
    for k in range(NCORES):
        sh = k * SH
        ys_r, ysa_r, yt_r, yta_r = [np.roll(a, -sh, axis=0) for a in arrs]
        lab_r = np.roll(lab, -sh)
        in_maps.append({
            "yT_sx": np.ascontiguousarray(np.concatenate([ys_r.T, ysa_r.T], 0)),
            "yT_tx": np.ascontiguousarray(np.concatenate([yt_r.T, yta_r.T], 0)),
            "yta_nat": np.ascontiguousarray(
                yta_r.reshape(8, 128, 64).transpose(1, 0, 2).reshape(128, 512)),
            "y4_nat": np.ascontiguousarray(
                np.concatenate([ys_r[:SH], ysa_r[:SH], yt_r[:SH], yta_r[:SH]], 1)),
            "lab_row": np.ascontiguousarray(lab_r.reshape(1, N)),
            "lab_sh": np.ascontiguousarray(lab_r[:SH].reshape(SH, 1)),
            "iota_mod": iota_mod,
            "ident": ident,
            "lhA": lhA, "lhB": lhB, "lbS": lbS, "lbT": lbT,
        })
    return in_maps


def kernel(y_s, y_s_adv, labels_s, y_t, y_t_adv, epoch, _trace=False):
    beta_t = 1.0 if int(np.asarray(epoch)) >= 10 else 0.0
    nc = _get_nc(beta_t)
    in_maps = make_in_maps(y_s, y_s_adv, labels_s, y_t, y_t_adv)
    res = run_bass_kernel_spmd(nc, in_maps, core_ids=list(range(NCORES)),
                               trace=_trace)
    tot = np.zeros(2, dtype=np.float64)
    for r in res.results:
        tot += r["out2"].reshape(2).astype(np.float64)
    out = (np.float32(tot[0]), np.float32(tot[1]))
    if _trace:
        return out, res
    return out


# revision 12
# speedup vs baseline: 1.5301x; 1.5301x over previous
"""Trainium2 Bass kernel for nn_AUCDomainAdapation (AUC domain-adaptation loss).

Contract: kernel(**inputs) takes the FULL unsharded inputs of reference.setup_inputs()
and returns the same structure as reference.reference(**inputs): a tuple
(0.25*empirical, transfer) of float32 scalars.

Math (validated vs reference):
  For sample i with label c_i, the masked pairwise sum per term reduces to
    sum_j M_ij * L(Q_ij),  Q_ij = q0_i + s * B'_ij
  where B'_ij = (s*Y_shard @ pTm)_ij and pTm[c, j] = p[j, c] * [label_j != c]
  (class-major softmax probs with same-label columns zeroed).  Masked entries
  hit B' = 0 exactly, so the unmasked row-sum minus n_{c_i} * L(q0_i) is the
  masked sum.  L(Q) = softplus(-Q) + softplus(Q + 2*eps)
              = ln((1 + e^{2 eps}) + e^{Q+2 eps} + e^{-Q}).

Sharding: row-shard the 1024 samples over 8 cores (128 rows each).  Inputs are
rotated per-core along the sample axis so a single SPMD program (slice [0:128])
serves all cores; per-core scalar partials are summed on the host.
"""

import numpy as np
from contextlib import ExitStack

import concourse.bass as bass
import concourse.bacc as bacc
import concourse.tile as tile
from concourse import mybir
from concourse.bass_utils import run_bass_kernel_spmd
from concourse._compat import with_exitstack

F32 = mybir.dt.float32
BF16 = mybir.dt.bfloat16
AF = mybir.ActivationFunctionType
ALU = mybir.AluOpType

N = 1024          # samples
C = 64            # classes
SH = 128          # rows per core
NCORES = 8
EPS = 0.05
C0 = float(1.0 + np.exp(2 * EPS))   # 1 + e^{2eps}
SE = float(np.exp(EPS))             # e^{eps}
CHUNK = 512                         # matmul moving free-dim max

INPUT_SPECS = {
    "yT_sx":   ([128, N], BF16),   # [y_s^T ; y_s_adv^T] class-major stack
    "yT_tx":   ([128, N], BF16),   # [y_t^T ; y_t_adv^T]
    "yta_nat": ([128, 512], F32),  # y_t_adv natural, j=(a,p): [p, a*64+c]
    "y4_nat":  ([128, 256], F32),  # shard rows: [y_s | y_sa | y_t | y_ta]
    "lab_row": ([1, N], F32),      # labels_s as f32 row
    "lab_sh":  ([128, 1], F32),    # shard labels f32
    "iota_mod": ([128, 1], F32),   # p mod 64
    "ident":   ([128, 128], F32),  # identity matrix
    "lhA":     ([128, 4], BF16),   # colsum lhsT: source stack -> rows 0/1
    "lhB":     ([128, 4], BF16),   # colsum lhsT: target stack -> rows 2/3
    "lbS":     ([4, 128], BF16),   # rb broadcast lhsT (source halves)
    "lbT":     ([4, 128], BF16),   # rb broadcast lhsT (target halves)
}


def _patch_act_tables():
    """Force exp+ln to resolve to the single natural_log_exp_and_others set,
    avoiding table reloads between Exp and Ln activations."""
    if getattr(bacc, "_act_tables_patched", False):
        return
    orig = bacc.get_activation_tables

    def patched(arch):
        tabs = dict(orig(arch))
        out = {}
        for name, funcs in tabs.items():
            if name != "natural_log_exp_and_others":
                funcs = {f for f in funcs if f not in (AF.Exp, AF.Ln)}
            out[name] = funcs
        return out

    bacc.get_activation_tables = patched
    bacc._act_tables_patched = True


@with_exitstack
def _body(ctx: ExitStack, tc: "tile.TileContext", ins: dict, out2: "bass.AP", beta_t: float):
    nc = tc.nc
    sb = ctx.enter_context(tc.tile_pool(name="sb", bufs=1))
    sw = ctx.enter_context(tc.tile_pool(name="sw", bufs=2))   # loss scratch
    psT = ctx.enter_context(tc.tile_pool(name="psT", bufs=2, space="PSUM"))
    psP = ctx.enter_context(tc.tile_pool(name="psP", bufs=2, space="PSUM"))

    # ---- input loads ------------------------------------------------------
    t_ysx = sb.tile([128, N], BF16)
    t_ytx = sb.tile([128, N], BF16)
    t_ytan = sb.tile([128, 512], F32)
    t_y4 = sb.tile([128, 256], F32)
    t_lab1 = sb.tile([1, N], F32)
    t_labsh = sb.tile([128, 1], F32)
    t_iotam = sb.tile([128, 1], F32)
    t_ident = sb.tile([128, 128], F32)
    t_lhA = sb.tile([128, 4], BF16)
    t_lhB = sb.tile([128, 4], BF16)
    t_lbS = sb.tile([4, 128], BF16)
    t_lbT = sb.tile([4, 128], BF16)
    for t, name in ((t_ysx, "yT_sx"), (t_ytx, "yT_tx"), (t_ytan, "yta_nat"),
                    (t_y4, "y4_nat"), (t_lab1, "lab_row"), (t_labsh, "lab_sh"),
                    (t_iotam, "iota_mod"), (t_ident, "ident"),
                    (t_lhA, "lhA"), (t_lhB, "lhB"), (t_lbS, "lbS"), (t_lbT, "lbT")):
        nc.sync.dma_start(t[:], ins[name][:])

    # ---- small constants --------------------------------------------------
    t_epsb = sb.tile([128, 1], F32)
    nc.gpsimd.memset(t_epsb[:], EPS)
    t_c0b = sb.tile([128, 1], F32)
    nc.gpsimd.memset(t_c0b[:], C0)
    t_ones = sb.tile([128, 1], F32)
    nc.gpsimd.memset(t_ones[:], 1.0)

    # ---- source one-hot stack + counts ------------------------------------
    t_lab128 = sb.tile([128, N], F32)
    nc.gpsimd.partition_broadcast(t_lab128[:], t_lab1[0:1, :])
    t_YTsx = sb.tile([128, N], F32)
    t_ns = sb.tile([128, 1], F32)
    nc.vector.tensor_scalar(t_YTsx[:], t_lab128[:], t_iotam[:], None,
                            op0=ALU.is_equal, op1=ALU.add, accum_out=t_ns[:])

    # ---- exps of the class-major stacks (bf16) ------------------------------
    t_Exs = sb.tile([128, N], BF16)
    nc.scalar.activation(t_Exs[:], t_ysx[:], AF.Exp)
    t_Ext = sb.tile([128, N], BF16)
    nc.scalar.activation(t_Ext[:], t_ytx[:], AF.Exp)

    # ---- softmax denominators (column sums) & fast reciprocals --------------
    p_cs = psP.tile([4, N], F32, tag="prep")
    for ch in range(2):
        s = slice(ch * CHUNK, (ch + 1) * CHUNK)
        nc.tensor.matmul(p_cs[:, s], t_lhA[:], t_Exs[:, s], start=True, stop=False)
        nc.tensor.matmul(p_cs[:, s], t_lhB[:], t_Ext[:, s], start=False, stop=True)
    t_R4 = sb.tile([4, N], F32)
    nc.vector.reciprocal_approx_fast(t_R4[:], p_cs[:])
    t_R4b = sb.tile([4, N], BF16)
    nc.vector.tensor_copy(t_R4b[:], t_R4[:])

    # ---- masked-normalizer RM and masked probs psTm: source -----------------
    p_rbs = psP.tile([128, N], F32, tag="prep")
    for ch in range(2):
        s = slice(ch * CHUNK, (ch + 1) * CHUNK)
        nc.tensor.matmul(p_rbs[:, s], t_lbS[:], t_R4b[:, s], start=True, stop=True)
    t_RMs = sb.tile([128, N], BF16)
    nc.vector.scalar_tensor_tensor(t_RMs[:], t_YTsx[:], 0.0, p_rbs[:],
                                   op0=ALU.is_equal, op1=ALU.mult)
    t_PTs_lo = sb.tile([64, N], BF16)
    nc.vector.tensor_tensor(t_PTs_lo[:], t_Exs[0:64, :], t_RMs[0:64, :], op=ALU.mult)
    t_PTs_hi = sb.tile([64, N], BF16)
    nc.vector.tensor_tensor(t_PTs_hi[:], t_Exs[64:128, :], t_RMs[64:128, :], op=ALU.mult)

    # ---- target pseudo-labels (one-hot argmax of y_t_adv), class-major ------
    t_m8 = sb.tile([128, 8], F32)
    ytan_v = t_ytan[:].rearrange("p (a c) -> p a c", c=64)
    nc.vector.tensor_reduce(t_m8[:], ytan_v, axis=mybir.AxisListType.X, op=ALU.max)
    t_oh = sb.tile([128, 512], F32)
    nc.vector.tensor_tensor(t_oh[:].rearrange("p (a c) -> p a c", c=64), ytan_v,
                            t_m8[:, :, None].broadcast_to((128, 8, 64)),
                            op=ALU.is_equal)
    p_yt = psP.tile([64, N], F32, tag="prep")
    for a in range(8):
        nc.tensor.transpose(p_yt[:, a * 128:(a + 1) * 128],
                            t_oh[:, a * 64:(a + 1) * 64], t_ident[:])
    t_YtT = sb.tile([64, N], F32)
    t_nt = sb.tile([64, 1], F32)
    nc.vector.tensor_scalar(t_YtT[:], p_yt[:], 0.0, None,
                            op0=ALU.add, op1=ALU.add, accum_out=t_nt[:])

    # ---- masked probs psTm: target -------------------------------------------
    p_rbt = psP.tile([128, N], F32, tag="prep")
    for ch in range(2):
        s = slice(ch * CHUNK, (ch + 1) * CHUNK)
        nc.tensor.matmul(p_rbt[:, s], t_lbT[:], t_R4b[:, s], start=True, stop=True)
    t_RMt = sb.tile([128, N], BF16)
    nc.vector.scalar_tensor_tensor(t_RMt[0:64, :], t_YtT[:], 0.0, p_rbt[0:64, :],
                                   op0=ALU.is_equal, op1=ALU.mult)
    nc.vector.scalar_tensor_tensor(t_RMt[64:128, :], t_YtT[:], 0.0, p_rbt[64:128, :],
                                   op0=ALU.is_equal, op1=ALU.mult)
    t_PTt_lo = sb.tile([64, N], BF16)
    nc.vector.tensor_tensor(t_PTt_lo[:], t_Ext[0:64, :], t_RMt[0:64, :], op=ALU.mult)
    t_PTt_hi = sb.tile([64, N], BF16)
    nc.vector.tensor_tensor(t_PTt_hi[:], t_Ext[64:128, :], t_RMt[64:128, :], op=ALU.mult)

    # ---- per-class fac weights, gathered to shard rows ------------------------
    def fac_of(n_ap):
        t1 = sb.tile([64, 1], F32, tag="fac_t1")
        nc.vector.tensor_scalar(t1[:], n_ap, -1.0, float(N), op0=ALU.mult, op1=ALU.add)
        t2 = sb.tile([64, 1], F32, tag="fac_t2")
        nc.vector.tensor_tensor(t2[:], t1[:], n_ap, op=ALU.mult)   # n*(N-n)
        t3 = sb.tile([64, 1], F32, tag="fac_t3")
        nc.vector.tensor_scalar(t3[:], t2[:], 1.0, None, op0=ALU.max)
        rec = sb.tile([64, 1], F32, tag="fac_rec")
        nc.vector.reciprocal(rec[:], t3[:])
        g = sb.tile([64, 1], F32, tag="fac_g")
        nc.vector.tensor_scalar(g[:], t2[:], 0.5, None, op0=ALU.is_gt)
        fac = sb.tile([64, 1], F32, tag="fac_out")
        nc.vector.tensor_tensor(fac[:], rec[:], g[:], op=ALU.mult)
        return fac

    fac_s = fac_of(t_ns[0:64, :])
    fac_t = fac_of(t_nt[:])

    t_rhs_s = sb.tile([64, 4], F32)
    nc.vector.tensor_scalar(t_rhs_s[:, 0:1], fac_s[:], 0.25, None, op0=ALU.mult)
    nc.vector.tensor_scalar(t_rhs_s[:, 1:2], fac_s[:], -0.5, None, op0=ALU.mult)
    nc.vector.tensor_copy(t_rhs_s[:, 2:3], t_ns[0:64, :])
    nc.vector.tensor_copy(t_rhs_s[:, 3:4], t_ns[0:64, :])
    t_rhs_t = sb.tile([64, 2], F32)
    nc.vector.tensor_scalar(t_rhs_t[:, 0:1], fac_t[:], 0.25 * beta_t, None, op0=ALU.mult)
    nc.vector.tensor_copy(t_rhs_t[:, 1:2], t_nt[:])

    p_gs = psP.tile([128, 4], F32, tag="prep")
    nc.tensor.matmul(p_gs[:], t_YTsx[0:64, 0:SH], t_rhs_s[:], start=True, stop=True)
    t_W3 = sb.tile([128, 3], F32)
    t_N3 = sb.tile([128, 3], F32)
    nc.vector.tensor_copy(t_W3[:, 0:2], p_gs[:, 0:2])
    nc.vector.tensor_copy(t_N3[:, 0:2], p_gs[:, 2:4])
    p_gt = psP.tile([128, 2], F32, tag="prep")
    nc.tensor.matmul(p_gt[:], t_YtT[:, 0:SH], t_rhs_t[:], start=True, stop=True)
    nc.vector.tensor_copy(t_W3[:, 2:3], p_gt[:, 0:1])
    nc.vector.tensor_copy(t_N3[:, 2:3], p_gt[:, 1:2])

    # ---- per-row selected logits / softmax denoms (natural layout) -----------
    t_iotaf = sb.tile([128, 64], F32)
    nc.gpsimd.iota(t_iotaf[:], pattern=[[1, 64]], base=0, channel_multiplier=0,
                   allow_small_or_imprecise_dtypes=True)
    t_Ysn = sb.tile([128, 64], F32)
    nc.vector.tensor_scalar(t_Ysn[:], t_iotaf[:], t_labsh[:], None, op0=ALU.is_equal)
    t_Y4n = sb.tile([128, 256], F32)
    nc.vector.tensor_copy(t_Y4n[:, 0:64], t_Ysn[:])
    nc.vector.tensor_copy(t_Y4n[:, 64:128], t_Ysn[:])
    nc.vector.tensor_copy(t_Y4n[:, 128:192], t_oh[:, 0:64])
    nc.vector.tensor_copy(t_Y4n[:, 192:256], t_oh[:, 0:64])
    t_scr = sb.tile([128, 256], F32)
    nc.vector.tensor_tensor(t_scr[:], t_Y4n[:], t_y4[:], op=ALU.mult)
    t_sel4 = sb.tile([128, 4], F32)
    nc.vector.tensor_reduce(t_sel4[:], t_scr[:].rearrange("p (a c) -> p a c", c=64),
                            axis=mybir.AxisListType.X, op=ALU.add)
    # shard softmax reciprocals: transpose R4[:, 0:128] -> [128, 4]
    p_r4t = psP.tile([128, 4], F32, tag="prep")
    nc.tensor.transpose(p_r4t[:], t_R4[:, 0:SH], t_ident[0:4, 0:4])
    t_es4 = sb.tile([128, 4], F32)
    nc.scalar.activation(t_es4[:], t_sel4[:], AF.Exp)
    t_a4 = sb.tile([128, 4], F32)
    nc.vector.tensor_tensor(t_a4[:], t_es4[:], p_r4t[:], op=ALU.mult)

    # ---- per-term q0 row constants and Exp biases ------------------------------
    t_q0 = sb.tile([128, 3], F32)
    # emp: q0 = 4 - eps - 4 a
    nc.vector.tensor_scalar(t_q0[:, 0:1], t_a4[:, 0:1], -4.0, 4.0 - EPS,
                            op0=ALU.mult, op1=ALU.add)
    # src: q0 = 2 (aa - a) - eps
    t_u = sb.tile([128, 1], F32, tag="u1")
    nc.vector.tensor_tensor(t_u[:], t_a4[:, 1:2], t_a4[:, 0:1], op=ALU.subtract)
    nc.vector.tensor_scalar(t_q0[:, 1:2], t_u[:], 2.0, -EPS, op0=ALU.mult, op1=ALU.add)
    # tgt: q0 = 2 (ata - at) - eps
    t_u2 = sb.tile([128, 1], F32, tag="u2")
    nc.vector.tensor_tensor(t_u2[:], t_a4[:, 3:4], t_a4[:, 2:3], op=ALU.subtract)
    nc.vector.tensor_scalar(t_q0[:, 2:3], t_u2[:], 2.0, -EPS, op0=ALU.mult, op1=ALU.add)
    # biases: u-exp bias = q0 + 2*eps ; v-exp bias = -q0
    t_b2 = sb.tile([128, 3], F32)
    nc.vector.tensor_scalar(t_b2[:], t_q0[:], 2.0 * EPS, None, op0=ALU.add)
    t_bn = sb.tile([128, 3], F32)
    nc.vector.tensor_scalar(t_bn[:], t_q0[:], -1.0, None, op0=ALU.mult)

    # ---- L0 corrections (loss at B'=0) -----------------------------------------
    t_w0 = sb.tile([128, 3], F32)
    nc.scalar.activation(t_w0[:], t_q0[:], AF.Exp, bias=t_epsb[:], scale=1.0)
    t_wi0 = sb.tile([128, 3], F32)
    nc.vector.reciprocal(t_wi0[:], t_w0[:])
    t_z0 = sb.tile([128, 3], F32)
    nc.vector.tensor_tensor(t_z0[:], t_w0[:], t_wi0[:], op=ALU.add)
    t_L0 = sb.tile([128, 3], F32)
    nc.scalar.activation(t_L0[:], t_z0[:], AF.Ln, bias=t_c0b[:], scale=SE)
    t_corr = sb.tile([128, 3], F32)
    nc.vector.tensor_tensor(t_corr[:], t_L0[:], t_N3[:], op=ALU.mult)

    # ---- big per-term matmuls + loss (bf16 matmuls, f32 loss) -------------------
    t_lhemp = sb.tile([64, 128], BF16)
    nc.vector.tensor_scalar(t_lhemp[:], t_YTsx[0:64, 0:SH], 4.0, None, op0=ALU.mult)
    t_lh2s = sb.tile([64, 128], BF16)
    nc.vector.tensor_scalar(t_lh2s[:], t_YTsx[0:64, 0:SH], 2.0, None, op0=ALU.mult)
    t_lhm2s = sb.tile([64, 128], BF16)
    nc.vector.tensor_scalar(t_lhm2s[:], t_YTsx[0:64, 0:SH], -2.0, None, op0=ALU.mult)
    t_lh2t = sb.tile([64, 128], BF16)
    nc.vector.tensor_scalar(t_lh2t[:], t_YtT[:, 0:SH], 2.0, None, op0=ALU.mult)
    t_lhm2t = sb.tile([64, 128], BF16)
    nc.vector.tensor_scalar(t_lhm2t[:], t_YtT[:, 0:SH], -2.0, None, op0=ALU.mult)

    t_as = sb.tile([128, 3], F32)
    terms = [
        ("emp", t_lhemp, None, t_PTs_lo, None, 0),
        ("src", t_lh2s, t_lhm2s, t_PTs_lo, t_PTs_hi, 1),
        ("tgt", t_lh2t, t_lhm2t, t_PTt_lo, t_PTt_hi, 2),
    ]
    for name, lh_a, lh_b, pt_a, pt_b, col in terms:
        p_T = psT.tile([128, N], F32, tag="T")
        for ch in range(2):
            s = slice(ch * CHUNK, (ch + 1) * CHUNK)
            if lh_b is not None:
                nc.tensor.matmul(p_T[:, s], lh_a[:], pt_a[:, s], start=True, stop=False)
                nc.tensor.matmul(p_T[:, s], lh_b[:], pt_b[:, s], start=False, stop=True)
            else:
                nc.tensor.matmul(p_T[:, s], lh_a[:], pt_a[:, s], start=True, stop=True)
        # u = e^{Q+2eps}, v = e^{-Q}; z = u + v; L = ln(z + c0), accum rows
        t_eu = sw.tile([128, N], F32, tag="eu")
        nc.scalar.activation(t_eu[:], p_T[:], AF.Exp, bias=t_b2[:, col:col + 1], scale=1.0)
        t_ev = sw.tile([128, N], F32, tag="ev")
        nc.scalar.activation(t_ev[:], p_T[:], AF.Exp, bias=t_bn[:, col:col + 1], scale=-1.0)
        t_z = sw.tile([128, N], F32, tag="z")
        nc.gpsimd.tensor_tensor(t_z[:], t_eu[:], t_ev[:], op=ALU.add)
        t_l = sw.tile([128, N], F32, tag="l")
        nc.scalar.activation(t_l[:], t_z[:], AF.Ln, bias=t_c0b[:], scale=1.0,
                             accum_out=t_as[:, col:col + 1])

    # ---- final reduction ---------------------------------------------------------
    t_r3 = sb.tile([128, 3], F32)
    nc.vector.tensor_tensor(t_r3[:], t_as[:], t_corr[:], op=ALU.subtract)
    t_wr3 = sb.tile([128, 3], F32)
    nc.vector.tensor_tensor(t_wr3[:], t_W3[:], t_r3[:], op=ALU.mult)
    p_fin = psP.tile([1, 3], F32, tag="prep")
    nc.tensor.matmul(p_fin[:], t_ones[:], t_wr3[:], start=True, stop=True)
    t_fin = sb.tile([1, 3], F32)
    nc.vector.tensor_copy(t_fin[:], p_fin[:])
    t_o = sb.tile([1, 2], F32)
    nc.vector.tensor_copy(t_o[:, 0:1], t_fin[:, 0:1])
    nc.vector.tensor_tensor(t_o[:, 1:2], t_fin[:, 1:2], t_fin[:, 2:3], op=ALU.add)
    nc.sync.dma_start(out2[:], t_o[:])


_NC_CACHE = {}


def _get_nc(beta_t: float):
    key = beta_t
    if key in _NC_CACHE:
        return _NC_CACHE[key]
    _patch_act_tables()
    nc = bacc.Bacc("TRN2", target_bir_lowering=False, debug=False, num_devices=NCORES)
    ins = {name: nc.dram_tensor(name, shape, dt, kind="ExternalInput").ap()
           for name, (shape, dt) in INPUT_SPECS.items()}
    out2 = nc.dram_tensor("out2", [1, 2], F32, kind="ExternalOutput").ap()
    with tile.TileContext(nc) as tc:
        _body(tc, ins, out2, beta_t)
    nc.compile()
    _NC_CACHE[key] = nc
    return nc


def make_in_maps(y_s, y_s_adv, labels_s, y_t, y_t_adv):
    bf16 = mybir.dt.np(BF16)
    lab = np.asarray(labels_s).astype(np.float32)
    iota_mod = (np.arange(128) % 64).astype(np.float32).reshape(128, 1)
    ident = np.eye(128, dtype=np.float32)
    lhA = np.zeros((128, 4), bf16); lhA[0:64, 0] = 1.0; lhA[64:128, 1] = 1.0
    lhB = np.zeros((128, 4), bf16); lhB[0:64, 2] = 1.0; lhB[64:128, 3] = 1.0
    lbS = np.zeros((4, 128), bf16); lbS[0, 0:64] = 1.0; lbS[1, 64:128] = 1.0
    lbT = np.zeros((4, 128), bf16); lbT[2, 0:64] = 1.0; lbT[3, 64:128] = 1.0
    arrs = [np.ascontiguousarray(np.asarray(a, dtype=np.float32))
            for a in (y_s, y_s_adv, y_t, y_t_adv)]
    in_maps = []
    for k in range(NCORES):
        sh = k * SH
        ys_r, ysa_r, yt_r, yta_r = [np.roll(a, -sh, axis=0) for a in arrs]
        lab_r = np.roll(lab, -sh)
        in_maps.append({
            "yT_sx": np.ascontiguousarray(
                np.concatenate([ys_r.T, ysa_r.T], 0).astype(bf16)),
            "yT_tx": np.ascontiguousarray(
                np.concatenate([yt_r.T, yta_r.T], 0).astype(bf16)),
            "yta_nat": np.ascontiguousarray(
                yta_r.reshape(8, 128, 64).transpose(1, 0, 2).reshape(128, 512)),
            "y4_nat": np.ascontiguousarray(
                np.concatenate([ys_r[:SH], ysa_r[:SH], yt_r[:SH], yta_r[:SH]], 1)),
            "lab_row": np.ascontiguousarray(lab_r.reshape(1, N)),
            "lab_sh": np.ascontiguousarray(lab_r[:SH].reshape(SH, 1)),
            "iota_mod": iota_mod,
            "ident": ident,
            "lhA": lhA, "lhB": lhB, "lbS": lbS, "lbT": lbT,
        })
    return in_maps


def kernel(y_s, y_s_adv, labels_s, y_t, y_t_adv, epoch, _trace=False):
    beta_t = 1.0 if int(np.asarray(epoch)) >= 10 else 0.0
    nc = _get_nc(beta_t)
    in_maps = make_in_maps(y_s, y_s_adv, labels_s, y_t, y_t_adv)
    res = run_bass_kernel_spmd(nc, in_maps, core_ids=list(range(NCORES)),
                               trace=_trace)
    tot = np.zeros(2, dtype=np.float64)
    for r in res.results:
        tot += r["out2"].reshape(2).astype(np.float64)
    out = (np.float32(tot[0]), np.float32(tot[1]))
    if _trace:
        return out, res
    return out
